# revision 30
# baseline (speedup 1.0000x reference)
"""Trainium2 Bass kernel for AttentiveGraphConvolutionSparse (GAT-style layer).

Computation (see reference):
    h   = x @ W                         [N, D_OUT]
    a_s = h @ attn_self                 [N, 1]
    a_n = h @ attn_neigh                [N, 1]
    e   = leaky_relu(a_s + a_n.T, 0.2)  [N, N]
    e  += MASK_VAL * (1 - adj)
    attn = softmax(e, axis=-1)
    out = relu(attn @ h)

Sharding: row-parallel over queries (core c owns queries [c*R, (c+1)*R)).

Transposed-logits design: the HOST ships each core adj^T for its query
columns as fp16 mask OFFSETS adjP[j, grp, q] = BIG*(adj[i,j]-1) in {-BIG, 0},
with j rows permuted into gather-chunk order.  On-chip tiles then carry j on
PARTITIONS and i on the free axis, so:
  - a_n[j] is the custom-DVE op's per-partition scalar (no N-wide broadcast)
  - a_s is one small [P, R] broadcast built once in the prologue
  - z = leaky(asb + a_n) + adjP   (one fused DVE pass)
  - s = exp(z)  bf16              (one ACT pass; masked lanes give e^-~40*? ~ 0)
  - s IS the matmul lhsT directly: out[i,:] += s[:, i-tile].T @ [h | 1 | 1]
    -- no PE transposes, no PSUM->SBUF copies, denominator rides the ones cols.
The i-range is split in 2 groups of 6 i-tiles so the six 258-col f32
accumulators fit 6 PSUM banks across the 96-deep j accumulation.

h | ones | a_n(hi,lo) are computed from the local x slice and exchanged with
3 pipelined chunk AllGathers (512 local rows each) so the first j-tiles are
usable ~10us after the first h tiles; gather-order == adjP row order by
construction, so the program stays uniform across cores.
"""

import os

import numpy as np

import concourse.bacc as bacc
import concourse.bass as bass
import concourse.mybir as mybir
import concourse.tile as tile
from concourse import masks
from concourse.bass_utils import run_bass_kernel_spmd
from concourse.mybir import ActivationFunctionType as AFT, AluOpType as ALU

N = 12288
D_IN = 512
D_OUT = 256
NCORES = 8
R = N // NCORES          # 1536 query rows per core
P = 128
NIT = R // P             # 12 i-tiles per core
NJT = N // P             # 96 j-tiles
NGRP = 2
GIT = NIT // NGRP        # 6 i-tiles per group
GW = GIT * P             # 768 i columns per group
CHUNKS = (4, 4, 4)        # gather chunk sizes in i-tiles
NCH = len(CHUNKS)
CHOFF = tuple(sum(CHUNKS[:i]) for i in range(NCH + 1))
JPAIR = 2                # j-tiles per adj DMA
HCOL = D_OUT + 4         # h | one | one | an_hi | an_lo   (260)
BIG = 200.0
ALPHA = 0.2
F32 = mybir.dt.float32
F16 = mybir.dt.float16
BF16 = mybir.dt.bfloat16

_cache = {}
last_results = None


def _register_leaky_mask():
    """Custom DVE op: out = max(y, s1*y)*imm2 + in1, with y = in0 + s0.

    Used with imm2=1: z = leaky_relu(asb + a_n, alpha) + adjP, where adjP is
    the pre-scaled host-side mask offset in {-BIG, 0}.
    """
    from concourse import dve_ops as dops
    from concourse.dve_spec import (
        C0, C1, C2, Spec, Src0, Src1, lower as dve_lower, maxx,
    )
    from concourse.dve_uop import DveOpSpec

    name = "LEAKY_MASK_ANT"
    for op in dops.OPS:
        if op.name == name:
            return op
    _y = Src0 + C0
    spec = Spec(
        body=maxx(_y, _y * C1) * C2 + Src1,
        reference=lambda in0, in1, s0, s1, imm2: (
            np.maximum(in0.astype(np.float32) + s0,
                       (in0.astype(np.float32) + s0) * s1) * imm2 + in1
        ).astype(np.float32),
    )
    row = dops._CUSTOM_DVE_ROW_BASE + len(dops.OPS)
    assert row < 0x20
    sha = {}
    for ver in ("v3", "v4"):
        s = DveOpSpec(name=name, opcode=row, uops=dve_lower(spec, ver=ver),
                      rd1_en=True)
        sha[ver] = s.sha(ver)
    op = dops.DveOp(name, spec, subdim=False, uops_sha=sha)
    dops.OPS.append(op)
    dops._SUB_OPCODE_FOR_NAME[name] = row
    dops.CUSTOM_DVE_SPECS[name] = spec
    return op


def _install_ntff_hook():
    """Register the axon NTFF profiling hook (missing antenv.axon_hooks shim)."""
    import contextlib
    import ctypes
    import sys
    import types

    if "antenv.axon_hooks" in sys.modules:
        return
    so_path = "/opt/axon/libaxon_pjrt.so"
    if not os.path.exists(so_path):
        return
    lib = ctypes.CDLL(so_path)
    if not hasattr(lib, "axon_start_nrt_profile"):
        return
    lib.axon_start_nrt_profile.argtypes = [ctypes.POINTER(ctypes.c_int64),
                                           ctypes.c_size_t]
    lib.axon_start_nrt_profile.restype = ctypes.c_int64
    lib.axon_stop_nrt_profile.argtypes = [ctypes.c_char_p]
    lib.axon_stop_nrt_profile.restype = ctypes.c_int64

    @contextlib.contextmanager
    def _hook(output_dir, device_ids):
        import jax
        jax.devices()
        if device_ids:
            ids = (ctypes.c_int64 * len(device_ids))(*device_ids)
            rc = lib.axon_start_nrt_profile(ids, len(device_ids))
        else:
            rc = lib.axon_start_nrt_profile(None, 0)
        if rc != 0:
            raise RuntimeError(f"axon_start_nrt_profile rc={rc}")
        try:
            yield
        finally:
            n = lib.axon_stop_nrt_profile(str(output_dir).encode())
            print(f"profile: {n} file(s) written to {output_dir}",
                  file=sys.stderr)

    _state = {"hook": _hook}
    mod = types.ModuleType("antenv.axon_hooks")
    mod.get_axon_ntff_profile_hook = lambda: _state["hook"]

    def _set(h):
        _state["hook"] = h

    mod.set_axon_ntff_profile_hook = _set
    sys.modules["antenv.axon_hooks"] = mod


def _build():
    global _LEAKY_MASK
    _LEAKY_MASK = _register_leaky_mask()
    nc = bacc.Bacc("TRN2", target_bir_lowering=False, debug=False,
                   num_devices=NCORES)

    x_p = nc.declare_dram_parameter("x", [R, D_IN], F16, isOutput=False).ap()
    w_p = nc.declare_dram_parameter("W", [D_IN, D_OUT], F16, isOutput=False).ap()
    as_p = nc.declare_dram_parameter("attn_self", [D_OUT, 1], F32, isOutput=False).ap()
    an_p = nc.declare_dram_parameter("attn_neigh", [D_OUT, 1], F32, isOutput=False).ap()
    adj_p = nc.declare_dram_parameter("adjP", [N, NGRP, GW], F16, isOutput=False).ap()
    out_p = nc.declare_dram_parameter("out", [R, D_OUT], F32, isOutput=True).ap()

    h_loc = nc.dram_tensor("h_loc", [R, HCOL], BF16).ap()
    h_all = nc.dram_tensor("h_all", [N, HCOL], BF16, addr_space="Shared").ap()
    as_scr = nc.dram_tensor("as_scr", [R, 1], F32).ap()
    warm_loc = nc.dram_tensor("warm_loc", [1, 1], F32).ap()
    warm_all = nc.dram_tensor("warm_all", [NCORES, 1], F32,
                              addr_space="Shared").ap()

    from contextlib import ExitStack

    grp_all = [list(range(NCORES))]

    with tile.TileContext(nc) as tc, ExitStack() as octx:
        const = octx.enter_context(tc.tile_pool(name="const", bufs=1))
        ident = const.tile([P, P], F32)
        masks.make_identity(nc, ident[:])
        ident_h = const.tile([P, P], F16)
        masks.make_identity(nc, ident_h[:])

        h_big = const.tile([P, NJT * HCOL], BF16)
        h_big_v = h_big[:].rearrange("p (jt c) -> p jt c", jt=NJT)
        h_all_v = h_all.rearrange("(jt p) c -> p jt c", p=P)
        asb = const.tile([P, R], F32)
        an_sb = const.tile([P, NJT], F32)
        as_sb = const.tile([P, NIT], F32)

        prsb = octx.enter_context(tc.tile_pool(name="prsb", bufs=2))

        # ---------------- prologue: h | 1 | 1 | a_n, a_s ----------------
        with tc.tile_pool(name="prol", bufs=1) as prol, \
             tc.tile_pool(name="prps", bufs=2, space="PSUM") as prps:
            # x arrives fp16 from the host in 3 separate piece tiles so the
            # first transposes only wait on piece 0's DMA
            XPC = 4
            xs = x_p.rearrange("(it p) k -> p it k", p=P)
            xch = []
            for pc in range(NIT // XPC):
                xc = prol.tile([P, XPC * D_IN], F16, name=f"x{pc}")
                nc.sync.dma_start(
                    out=xc[:].rearrange("p (it k) -> p it k", it=XPC),
                    in_=xs[:, pc * XPC:(pc + 1) * XPC, :])
                xch.append(xc)

            def xslice(it, kb):
                return xch[it // XPC][:, (it % XPC) * D_IN + kb * P:
                                      (it % XPC) * D_IN + (kb + 1) * P]
            wnat = prol.tile([P, 4 * D_OUT], F16)
            nc.sync.dma_start(
                out=wnat[:].rearrange("p (kb d) -> p kb d", kb=4),
                in_=w_p.rearrange("(kb p) d -> p kb d", p=P))
            asn = prol.tile([P, 4], F32)
            nc.sync.dma_start(
                out=asn[:, 0:2].rearrange("p (db one) -> p db one", one=1),
                in_=as_p.rearrange("(db p) one -> p db one", p=P))
            nc.sync.dma_start(
                out=asn[:, 2:4].rearrange("p (db one) -> p db one", one=1),
                in_=an_p.rearrange("(db p) one -> p db one", p=P))
            asn16 = prol.tile([P, 4], F16)
            nc.vector.tensor_copy(asn16[:], asn[:])

            # W^T tiles (fp16): col db*D_IN + kb*P + k
            wT = prol.tile([P, 2 * D_IN], F16)
            for db in range(2):
                pst = prps.tile([P, 4 * P], F16, tag="pst", bufs=2)
                for kb in range(4):
                    nc.tensor.transpose(
                        pst[:, kb * P:(kb + 1) * P],
                        wnat[:, kb * D_OUT + db * P: kb * D_OUT + (db + 1) * P],
                        ident_h[:])
                nc.vector.tensor_copy(
                    wT[:, db * D_IN:(db + 1) * D_IN], pst[:])

            # wext cols per kb: [ W_kb (256) | w_s_kb | w_n_kb ]  (258 wide)
            wext = prol.tile([P, 4 * (D_OUT + 2)], F16)
            for kb in range(4):
                nc.scalar.copy(
                    wext[:, kb * (D_OUT + 2): kb * (D_OUT + 2) + D_OUT],
                    wnat[:, kb * D_OUT: (kb + 1) * D_OUT])
            wps = prps.tile([P, 8], F32, tag="wps", bufs=1)
            for v in range(2):
                for kb in range(4):
                    for db in range(2):
                        nc.tensor.matmul(
                            wps[:, v * 4 + kb: v * 4 + kb + 1],
                            wT[:, db * D_IN + kb * P: db * D_IN + (kb + 1) * P],
                            asn16[:, 2 * v + db: 2 * v + db + 1],
                            start=(db == 0), stop=(db == 1))
                    nc.vector.tensor_copy(
                        wext[:, kb * (D_OUT + 2) + D_OUT + v:
                              kb * (D_OUT + 2) + D_OUT + v + 1],
                        wps[:, v * 4 + kb: v * 4 + kb + 1])

            # x^T tiles (fp16): col kb*R + it*P + i
            xT = prol.tile([P, 4 * R], F16)
            ci = 0
            for itq in range(NIT // 2):
                for kb in range(4):
                    pst = prps.tile([P, 2 * P], F16, tag="pst2", bufs=2)
                    for i2 in range(2):
                        it = itq * 2 + i2
                        nc.tensor.transpose(
                            pst[:, i2 * P:(i2 + 1) * P],
                            xslice(it, kb),
                            ident_h[:])
                    dst = xT[:].rearrange("p (kb it i) -> p kb it i",
                                          kb=4, it=NIT)[:, kb,
                                                        itq * 2:(itq + 1) * 2, :]
                    if ci % 2 == 0:
                        nc.vector.tensor_copy(dst, pst[:].rearrange(
                            "p (it i) -> p it i", it=2))
                    else:
                        nc.scalar.copy(dst, pst[:].rearrange(
                            "p (it i) -> p it i", it=2))
                    ci += 1

            # ones pre-set in cols 256/257 of every it block (denominator)
            hsb = prol.tile([P, NIT * HCOL], BF16)
            hv = hsb[:].rearrange("p (it c) -> p it c", it=NIT)
            nc.gpsimd.memset(hv[:, :, D_OUT:D_OUT + 2], 1.0)

            for it in range(NIT):
                hps = prps.tile([P, D_OUT + 2], F32, tag="hps", bufs=2)
                for kb in range(4):
                    nc.tensor.matmul(
                        hps[:],
                        xT[:, kb * R + it * P: kb * R + (it + 1) * P],
                        wext[:, kb * (D_OUT + 2): (kb + 1) * (D_OUT + 2)],
                        start=(kb == 0), stop=(kb == 3))
                dst_h = hsb[:, it * HCOL: it * HCOL + D_OUT]
                if it % 2 == 0:
                    nc.scalar.copy(dst_h, hps[:, 0:D_OUT])
                else:
                    nc.vector.tensor_copy(dst_h, hps[:, 0:D_OUT])
                nc.vector.tensor_copy(as_sb[:, it:it + 1],
                                      hps[:, D_OUT:D_OUT + 1])
                # a_n -> bf16 hi + lo (lo = a_n - f32(hi)) in cols 258/259
                nc.vector.tensor_copy(
                    hsb[:, it * HCOL + D_OUT + 2: it * HCOL + D_OUT + 3],
                    hps[:, D_OUT + 1:D_OUT + 2])
                nc.vector.scalar_tensor_tensor(
                    out=hsb[:, it * HCOL + D_OUT + 3: it * HCOL + D_OUT + 4],
                    in0=hsb[:, it * HCOL + D_OUT + 2: it * HCOL + D_OUT + 3],
                    scalar=-1.0,
                    in1=hps[:, D_OUT + 1:D_OUT + 2],
                    op0=ALU.mult, op1=ALU.add)

                if it + 1 in CHOFF[1:]:
                    ch = CHOFF.index(it + 1) - 1
                    lo, hi = CHOFF[ch], CHOFF[ch + 1]
                    nc.scalar.dma_start(
                        out=h_loc[lo * P:hi * P, :]
                            .rearrange("(it p) c -> p it c", p=P),
                        in_=hv[:, lo:hi, :])
                    nc.gpsimd.collective_compute(
                        "AllGather", ALU.bypass, replica_groups=grp_all,
                        ins=[h_loc[lo * P:hi * P, :]],
                        outs=[h_all[lo * NCORES * P:hi * NCORES * P, :]])

            # a_s broadcast row: transpose, DRAM roundtrip, broadcast load.
            # Emitted on the SWDGE FIFO BEFORE the h_big loads: asb is ready
            # ~25us (local data) while h_big waits on the gathers — keeping
            # it first avoids head-of-line blocking the main loop's customs.
            asT = prps.tile([P, P], F32, tag="asT", bufs=1)
            nc.tensor.transpose(asT[:NIT, :], as_sb[:], ident[:])
            ast = prsb.tile([NIT, P], F32, tag="ast")
            nc.vector.tensor_copy(ast[:], asT[:NIT, :])
            nc.scalar.dma_start(
                out=as_scr[:, 0:1].rearrange("(t p) one -> t (p one)", t=NIT),
                in_=ast[:])
            as_bsrc = as_scr.rearrange("n one -> one n").to_broadcast((P, R))
            nc.gpsimd.dma_start(out=asb[:], in_=as_bsrc)

            # gathered h -> SBUF in 8-j-tile sub-loads so the first
            # consumers unblock ~2us after each gather instead of waiting
            # a whole 2.1MB chunk (pure-DMA gpsimd queue; a_n adds live in
            # the main loop on Vector)
            for sub in range(NJT // NCORES):
                jlo, jhi = sub * NCORES, (sub + 1) * NCORES
                nc.gpsimd.dma_start(
                    out=h_big_v[:, jlo:jhi, :],
                    in_=h_all_v[:, jlo:jhi, :])

        # ---------------- main loop ----------------
        zpool = octx.enter_context(tc.tile_pool(name="zpool", bufs=6))
        zfpool = octx.enter_context(tc.tile_pool(name="zfpool", bufs=4))
        snpool = octx.enter_context(tc.tile_pool(name="snpool", bufs=4))
        opsum = octx.enter_context(tc.tile_pool(name="opsum", bufs=1, space="PSUM"))
        fpool = octx.enter_context(tc.tile_pool(name="fpool", bufs=2))

        adj_v = adj_p.rearrange("(jp two p) g w -> p jp two g w",
                                two=JPAIR, p=P)

        for grp in range(NGRP):
            outs = [opsum.tile([P, D_OUT + 2], F32, name=f"o{grp}_{i6}",
                               tag=f"outp{i6}") for i6 in range(GIT)]
            for jp in range(NJT // JPAIR):
                zt = zpool.tile([P, JPAIR * GW], F16)
                nc.sync.dma_start(
                    out=zt[:].rearrange("p (two w) -> p two w", two=JPAIR),
                    in_=adj_v[:, jp, :, grp, :])
                for half in range(JPAIR):
                    jt = jp * JPAIR + half
                    if grp == 0 and jt % NCORES == 0:
                        # a_n = hi + lo for the 8-j-tile sub-chunk starting
                        # here (Vector, right before its first consumer)
                        nc.vector.tensor_tensor(
                            out=an_sb[:, jt:jt + NCORES]
                                .rearrange("p (t one) -> p t one", one=1),
                            in0=h_big_v[:, jt:jt + NCORES,
                                        D_OUT + 2:D_OUT + 3],
                            in1=h_big_v[:, jt:jt + NCORES,
                                        D_OUT + 3:D_OUT + 4],
                            op=ALU.add)
                    zf = zfpool.tile([P, GW], F32, tag=f"zf{half}")
                    nc.vector._custom_dve(
                        _LEAKY_MASK, out=zf[:],
                        in0=asb[:, grp * GW:(grp + 1) * GW],
                        in1=zt[:, half * GW:(half + 1) * GW],
                        s0=an_sb[:, jt:jt + 1], s1=ALPHA, imm2=1.0)
                    sn = snpool.tile([P, GW], BF16, tag=f"sn{half}")
                    nc.scalar.activation(out=sn[:], in_=zf[:], func=AFT.Exp,
                                         bias=0.0, scale=1.0)
                    for i6 in range(GIT):
                        nc.tensor.matmul(
                            outs[i6][:],
                            sn[:, i6 * P:(i6 + 1) * P],
                            h_big[:, jt * HCOL: jt * HCOL + D_OUT + 2],
                            start=(jt == 0), stop=(jt == NJT - 1))
            # epilogue: relu(out * 1/denom) as ONE ACT pass (scale = per-
            # partition reciprocal); keeps the busy Vector engine to 12 tiny
            # reciprocals
            of = fpool.tile([P, GIT * D_OUT], F32, tag="of")
            for i6 in range(GIT):
                rec = fpool.tile([P, 1], F32, tag="rec")
                nc.vector.reciprocal(rec[:], outs[i6][:, D_OUT:D_OUT + 1])
                nc.scalar.activation(
                    out=of[:, i6 * D_OUT:(i6 + 1) * D_OUT],
                    in_=outs[i6][:, 0:D_OUT],
                    func=AFT.Relu, bias=0.0, scale=rec[:])
            nc.gpsimd.dma_start(
                out=out_p[grp * GW:(grp + 1) * GW, :]
                    .rearrange("(i6 p) d -> p i6 d", p=P),
                in_=of[:].rearrange("p (i6 d) -> p i6 d", i6=GIT))

    nc.compile()
    return nc


def _perm():
    """Gathered-row order: chunk ch gathers local rows [CHOFF[ch]*P,
    CHOFF[ch+1]*P) from every core, concatenated in rank order."""
    perm = np.empty(N, dtype=np.int64)
    for ch in range(NCH):
        o0, o1 = CHOFF[ch] * P, CHOFF[ch + 1] * P
        nloc = o1 - o0
        g0 = NCORES * o0
        for d in range(NCORES):
            perm[g0 + d * nloc: g0 + (d + 1) * nloc] = d * R + np.arange(o0, o1)
    return perm


def kernel(x, W, attn_self, attn_neigh, adj):
    global last_results
    if "nc" not in _cache:
        _cache["nc"] = _build()
    nc = _cache["nc"]

    x16 = np.ascontiguousarray(np.asarray(x, dtype=np.float16))
    W16 = np.ascontiguousarray(np.asarray(W, dtype=np.float16))
    attn_self = np.ascontiguousarray(np.asarray(attn_self, dtype=np.float32))
    attn_neigh = np.ascontiguousarray(np.asarray(attn_neigh, dtype=np.float32))
    adj = np.asarray(adj, dtype=np.float32)

    # adjm[i, j] = BIG*(adj[i,j]-1) in {-BIG, 0}; ship transposed [j, i] with
    # j rows in gathered order and i split into NGRP groups.
    adjm = ((adj - 1.0) * BIG).astype(np.float16)
    adjmT = np.ascontiguousarray(adjm.T)
    perm = _perm()
    adjmT = adjmT[perm]

    in_maps = []
    for c in range(NCORES):
        sl = slice(c * R, (c + 1) * R)
        adjP = np.ascontiguousarray(adjmT[:, sl]).reshape(N, NGRP, GW)
        in_maps.append({
            "x": np.ascontiguousarray(x16[sl]),
            "W": W16,
            "attn_self": attn_self,
            "attn_neigh": attn_neigh,
            "adjP": adjP,
        })

    trace = bool(os.environ.get("KERNEL_TRACE"))
    if trace:
        _install_ntff_hook()
    res = run_bass_kernel_spmd(nc, in_maps, list(range(NCORES)), trace=trace)
    last_results = res
    return np.concatenate([res.results[c]["out"] for c in range(NCORES)], axis=0)
